# revision 18
# baseline (speedup 1.0000x reference)
"""Bass/Trainium2 kernel for nn_CustomPooling (segment_reduce, masked mean pooling).

Reference computation:
  hs = mean(hidden_states[-4:], axis=0)                      # [B,S,H]
  valid = before_pad & ~CLS & ~SEP & attention
  term_mean = sum_s(hs * term_mask) / sum(term_mask)         # [B,H]
  text_mean = sum_s(hs * text_mask) / sum(text_mask)         # [B,H]
  out = concat([term_mean, text_mean], -1)                   # [B,2H]

Strategy:
  - Only the last 4 layers are ever read (201MB of the 654MB input).
  - The [B,S] int masks reduce to binary {0,1} per-(b,s) weights; the
    1/(4*count) scale is applied to the tiny [B,2H] result on the host, so
    the device work is a pure masked sum over (layer, s):
      acc[b, m*H + h] = sum_{l,s} hs[l,b,s,h] * mask[b,s,m]
  - That reduction is a TensorE matmul with the [128,2] binary mask slice
    stationary and hs [128, N] moving, accumulated in fp32 PSUM over
    4 s-chunks x 4 layers. Data is shipped as fp16 ({0,1} masks are exact;
    hs quantization gives ~4e-4 rel err) which halves DMA bytes and runs
    the PE at full (1 col/cycle) rate instead of the 4x-slower fp32 path.
  - Data parallel over B: 8 cores x 4 batches, no collectives.
  - Host pre-swizzles each (batch, layer-pair) into one contiguous
    [128, 6152] fp16 blob (its own weight copy appended) so each tile is
    ONE ~1.57MB DMA and every matmul waits on exactly one DMA semaphore
    (this toolchain accepts a single sync wait per instruction). The 8 hs
    DMAs alternate between the two HWDGE rings (sync/scalar) to keep all
    16 SDMA engines latency-hidden; the tiny output store uses SWDGE to
    avoid wrapping the 8 HWDGE semaphore lanes.
"""

import os

import numpy as np

# Hardcoded problem shape (kernel.py must be self-contained).
L, B, S, H = 13, 32, 512, 768
N_LAYERS = 4          # layers -4..-1
N_CORES = 8
B_SHARD = B // N_CORES          # 4 batches per core
N_CHUNKS = S // 128             # 4 s-chunks of 128 (PE contraction dim)
HS_COLS = N_CHUNKS * H                   # 3072 per (batch, layer) tile
W_COLS = N_CHUNKS * 2                    # 8
BLOB_COLS = HS_COLS + W_COLS             # 3080
CLS_ID, SEP_ID, PAD_ID = 101, 102, 0

_CACHED = {}


def _build_bass():
    import concourse.bass as bass
    import concourse.tile as tile
    from concourse import mybir

    f16 = mybir.dt.float16
    f32 = mybir.dt.float32
    nc = bass.Bass()

    # Per-core input: one blob per (batch, layer) + this batch's masks.
    #   blob[b, l, p, c*768 + h]   = fp16(hidden_states[9+l, B0+b, c*128+p, h])
    #   blob[b, l, p, 3072 + c*2 + m] = mask[B0+b, c*128+p, m]  (0/1)
    hs = nc.dram_tensor(
        "hs", [B_SHARD, N_LAYERS, 128, BLOB_COLS], f16, kind="ExternalInput"
    )
    out = nc.dram_tensor("out", [B_SHARD, 2 * H], f32, kind="ExternalOutput")

    with tile.TileContext(nc) as tc:
        with (
            tc.tile_pool(name="hs_pool", bufs=B_SHARD * N_LAYERS) as hs_pool,
            tc.tile_pool(name="out_pool", bufs=1) as out_pool,
            tc.tile_pool(name="psum", bufs=4, space="PSUM") as psum_pool,
        ):
            out_tile = out_pool.tile([2, B_SHARD * H], f32)

            for b in range(B_SHARD):
                psum_t = psum_pool.tile([2, H], f32, tag="psum")

                # One ~787KB DMA per (b, layer), alternating the HWDGE rings
                tiles = []
                for l in range(N_LAYERS):
                    t = hs_pool.tile([128, BLOB_COLS], f16, tag="hs")
                    eng = nc.sync if (b * N_LAYERS + l) % 2 == 0 else nc.scalar
                    eng.dma_start(out=t[:], in_=hs[b, l])
                    tiles.append(t)

                for l in range(N_LAYERS):
                    t = tiles[l]
                    for c in range(N_CHUNKS):
                        lhsT = t[:, HS_COLS + c * 2 : HS_COLS + c * 2 + 2]
                        col0 = c * H
                        rhs = t[:, col0 : col0 + H]
                        first = l == 0 and c == 0
                        last = l == N_LAYERS - 1 and c == N_CHUNKS - 1
                        # H=768 split at the 512-f32 PSUM bank boundary
                        nc.tensor.matmul(
                            psum_t[:, 0:512], lhsT, rhs[:, 0:512],
                            start=first, stop=last,
                        )
                        nc.tensor.matmul(
                            psum_t[:, 512:H], lhsT, rhs[:, 512:H],
                            start=first, stop=last,
                        )

                nc.vector.tensor_copy(
                    out=out_tile[:, b * H : (b + 1) * H], in_=psum_t[:, :]
                )

            # SBUF [2, (b h)] -> DRAM [b, (m h)] in one 24KB DMA.
            # SWDGE (gpsimd): the 8 hs DMAs consume all 8 HWDGE sem lanes;
            # a 9th HWDGE DMA would wrap the lane and need a 2nd sync wait.
            nc.gpsimd.dma_start(
                out=out.rearrange("b (m h) -> m b h", m=2),
                in_=out_tile[:].rearrange("m (b h) -> m b h", b=B_SHARD),
            )

    _fix_drain_waits(nc)
    return nc


def _fix_drain_waits(nc):
    """This container's walrus accepts only ONE sync wait per instruction;
    Tile's exit drain aggregates one wait per live semaphore. In this kernel
    every semaphore except the final out-DMA's is transitively ordered before
    the drain (matmuls wait on hs DMAs -> PE; copies wait on PE -> DVE; the
    out DMA waits on DVE; the drain runs after on the same SP queue), so the
    drain only truly needs the out-DMA completion wait.
    """
    import bass_rust

    f = nc.m.functions[0]
    # update-sem of the last DMACopy in program order (the out store)
    last_dma_sem = None
    for bb in f.blocks:
        for ins in bb.instructions:
            if type(ins).__name__ == "InstDMACopy":
                ups = ins.sync_info.on_update
                if ups:
                    last_dma_sem = ups[-1].ant_name

    for bb in f.blocks:
        for ins in bb.instructions:
            if type(ins).__name__ != "InstDrain":
                continue
            si = ins.sync_info
            if si is None:
                continue
            waits = list(si.on_wait)
            if len(waits) <= 1:
                continue
            keep = [w for w in waits if w.ant_name == last_dma_sem]
            assert len(keep) == 1, (last_dma_sem, [w.ant_name for w in waits])
            ins.sync_info = bass_rust.SyncInfo(
                on_wait=keep, on_update=list(si.on_update)
            )


def _host_masks(input_ids, attention_mask, token_type_ids):
    ids = np.asarray(input_ids)
    am = np.asarray(attention_mask)
    tt = np.asarray(token_type_ids)

    not_pad = ids != PAD_ID
    before_pad = np.cumprod(not_pad.astype(np.int64), axis=1).astype(bool)
    valid = before_pad & (ids != CLS_ID) & (ids != SEP_ID) & (am == 1)
    term = valid & (tt == 0)
    text = valid & (tt == 1)
    masks = np.stack([term, text], axis=-1)  # [B, S, 2] bool
    counts = masks.sum(axis=1).astype(np.float64)  # [B, 2]
    return masks.astype(np.float16), counts


def kernel(hidden_states, input_ids, attention_mask, token_type_ids):
    from concourse.bass_utils import run_bass_kernel_spmd

    hs_full = np.asarray(hidden_states)
    masks, counts = _host_masks(input_ids, attention_mask, token_type_ids)

    hs4 = hs_full[L - N_LAYERS :].astype(np.float16)  # [4, B, S, H]

    # Blob layout per (batch, layer): [p, (c h)] ++ [p, (c m)]
    blob = np.empty((B, N_LAYERS, 128, BLOB_COLS), dtype=np.float16)
    blob[:, :, :, :HS_COLS] = (
        hs4.reshape(N_LAYERS, B, N_CHUNKS, 128, H)
        .transpose(1, 0, 3, 2, 4)
        .reshape(B, N_LAYERS, 128, HS_COLS)
    )
    wv = masks.reshape(B, N_CHUNKS, 128, 2).transpose(0, 2, 1, 3).reshape(
        B, 128, W_COLS
    )
    blob[:, :, :, HS_COLS:] = wv[:, None, :, :]

    in_maps = [
        {"hs": blob[i * B_SHARD : (i + 1) * B_SHARD]} for i in range(N_CORES)
    ]

    if "nc" not in _CACHED:
        _CACHED["nc"] = _build_bass()
    nc = _CACHED["nc"]

    trace = os.environ.get("KERNEL_TRACE", "0") == "1"
    if trace:
        _install_ntff_hook_shim()
    tmpdir = os.environ.get("KERNEL_TMPDIR") or None
    res = run_bass_kernel_spmd(
        nc, in_maps, core_ids=list(range(N_CORES)), trace=trace, tmpdir=tmpdir
    )
    kernel.last_results = res

    acc = np.concatenate([r["out"] for r in res.results], axis=0)  # [B, 2H]
    # Apply the masked-mean normalization (exact f64 scale, mirrors the
    # reference's sum/count including inf/nan semantics for count==0).
    with np.errstate(divide="ignore", invalid="ignore"):
        scale = 1.0 / (N_LAYERS * counts)  # [B, 2]
    out = acc.reshape(B, 2, H) * scale[:, :, None]
    return out.reshape(B, 2 * H).astype(np.float32)


def _install_ntff_hook_shim():
    """The container's antenv stub lacks axon_hooks, which silently disables
    NTFF profiling under trace=True. Recreate it: a tiny get/set registry plus
    the ctypes hook into libaxon_pjrt.so (same as trn_boot's installer)."""
    import contextlib
    import ctypes
    import sys
    import types

    if "antenv.axon_hooks" in sys.modules:
        return
    so_path = "/opt/axon/libaxon_pjrt.so"
    try:
        lib = ctypes.CDLL(so_path)
    except OSError:
        return
    if not hasattr(lib, "axon_start_nrt_profile"):
        return
    lib.axon_start_nrt_profile.argtypes = [
        ctypes.POINTER(ctypes.c_int64),
        ctypes.c_size_t,
    ]
    lib.axon_start_nrt_profile.restype = ctypes.c_int64
    lib.axon_stop_nrt_profile.argtypes = [ctypes.c_char_p]
    lib.axon_stop_nrt_profile.restype = ctypes.c_int64

    @contextlib.contextmanager
    def _hook(output_dir, device_ids):
        import jax

        jax.devices()
        if device_ids:
            ids = (ctypes.c_int64 * len(device_ids))(*device_ids)
            rc = lib.axon_start_nrt_profile(ids, len(device_ids))
        else:
            rc = lib.axon_start_nrt_profile(None, 0)
        if rc != 0:
            raise RuntimeError(f"axon_start_nrt_profile rc={rc}")
        try:
            yield
        finally:
            n = lib.axon_stop_nrt_profile(str(output_dir).encode())
            print(f"profile: {n} file(s) written to {output_dir}", file=sys.stderr)

    mod = types.ModuleType("antenv.axon_hooks")
    _state = {"hook": _hook}
    mod.set_axon_ntff_profile_hook = lambda h: _state.__setitem__("hook", h)
    mod.get_axon_ntff_profile_hook = lambda: _state["hook"]
    sys.modules["antenv.axon_hooks"] = mod
    import antenv

    antenv.axon_hooks = mod


# revision 19
# speedup vs baseline: 1.1175x; 1.1175x over previous
"""Bass/Trainium2 kernel for nn_CustomPooling (segment_reduce, masked mean pooling).

Reference computation:
  hs = mean(hidden_states[-4:], axis=0)                      # [B,S,H]
  valid = before_pad & ~CLS & ~SEP & attention
  term_mean = sum_s(hs * term_mask) / sum(term_mask)         # [B,H]
  text_mean = sum_s(hs * text_mask) / sum(text_mask)         # [B,H]
  out = concat([term_mean, text_mean], -1)                   # [B,2H]

Strategy:
  - Only the last 4 layers are ever read (201MB of the 654MB input).
  - The [B,S] int masks reduce to binary {0,1} per-(b,s) weights; the
    1/(4*count) scale is applied to the tiny [B,2H] result on the host, so
    the device work is a pure masked sum over (layer, s):
      acc[b, m*H + h] = sum_{l,s} hs[l,b,s,h] * mask[b,s,m]
  - That reduction is a TensorE matmul with the [128,2] binary mask slice
    stationary and hs [128, N] moving, accumulated in fp32 PSUM over
    4 s-chunks x 4 layers. Data is shipped as fp16 ({0,1} masks are exact;
    hs quantization gives ~4e-4 rel err) which halves DMA bytes and runs
    the PE at full (1 col/cycle) rate instead of the 4x-slower fp32 path.
  - Data parallel over B: 8 cores x 4 batches, no collectives.
  - Host pre-swizzles each (batch, layer-pair) into one contiguous
    [128, 6152] fp16 blob (its own weight copy appended) so each tile is
    ONE ~1.57MB DMA and every matmul waits on exactly one DMA semaphore
    (this toolchain accepts a single sync wait per instruction). The 8 hs
    DMAs alternate between the two HWDGE rings (sync/scalar) to keep all
    16 SDMA engines latency-hidden; the tiny output store uses SWDGE to
    avoid wrapping the 8 HWDGE semaphore lanes.
"""

import os

import numpy as np

# Hardcoded problem shape (kernel.py must be self-contained).
L, B, S, H = 13, 32, 512, 768
N_LAYERS = 4          # layers -4..-1
N_CORES = 8
B_SHARD = B // N_CORES          # 4 batches per core
N_CHUNKS = S // 128             # 4 s-chunks of 128 (PE contraction dim)
W_COLS = N_CHUNKS * 2                    # 8
# Bulk batches (0..2) ship as two half-blobs (2 layers each); the tail
# batch ships as four quarter-blobs (1 layer) so the last-arriving tile
# only needs ~1.4us of matmuls after the final DMA lands.
HALF_HS = 2 * N_CHUNKS * H               # 6144
HALF_COLS = HALF_HS + W_COLS             # 6152
QUART_HS = N_CHUNKS * H                  # 3072
QUART_COLS = QUART_HS + W_COLS           # 3080
CLS_ID, SEP_ID, PAD_ID = 101, 102, 0

_CACHED = {}


def _build_bass():
    import concourse.bass as bass
    import concourse.tile as tile
    from concourse import mybir

    f16 = mybir.dt.float16
    f32 = mybir.dt.float32
    nc = bass.Bass()

    # Per-core inputs (host-preswizzled fp16 blobs, masks appended to each):
    #   hsa[b, hf, p, l2*3072 + c*768 + h], b in 0..2  (two half-blobs each)
    #   hsb[l, p, c*768 + h]                           (batch 3, per layer)
    hsa = nc.dram_tensor("hsa", [3, 2, 128, HALF_COLS], f16, kind="ExternalInput")
    hsb = nc.dram_tensor("hsb", [N_LAYERS, 128, QUART_COLS], f16, kind="ExternalInput")
    out = nc.dram_tensor("out", [B_SHARD, 2 * H], f32, kind="ExternalOutput")

    dma_idx = [0]

    def hs_dma(out_ap, in_ap):
        eng = nc.sync if dma_idx[0] % 2 == 0 else nc.scalar
        dma_idx[0] += 1
        eng.dma_start(out=out_ap, in_=in_ap)

    with tile.TileContext(nc) as tc:
        with (
            tc.tile_pool(name="hs_pool", bufs=6) as hs_pool,
            tc.tile_pool(name="hsq_pool", bufs=4) as hsq_pool,
            tc.tile_pool(name="out_pool", bufs=1) as out_pool,
            tc.tile_pool(name="psum", bufs=4, space="PSUM") as psum_pool,
        ):
            out_tile = out_pool.tile([2, B_SHARD * H], f32)

            for b in range(B_SHARD):
                # (lhsT, rhs_A, rhs_B) per (layer, chunk); weights live in
                # whichever tile the rhs comes from so each matmul waits on
                # exactly one DMA.
                mm_args = []
                if b < 3:
                    for hf in range(2):
                        t = hs_pool.tile([128, HALF_COLS], f16, tag="hs")
                        hs_dma(t[:], hsa[b, hf])
                        for l2 in range(2):
                            for c in range(N_CHUNKS):
                                lhsT = t[:, HALF_HS + c * 2 : HALF_HS + c * 2 + 2]
                                col0 = (l2 * N_CHUNKS + c) * H
                                mm_args.append((lhsT, t[:, col0 : col0 + 512],
                                                t[:, col0 + 512 : col0 + H]))
                else:
                    for l in range(N_LAYERS):
                        t = hsq_pool.tile([128, QUART_COLS], f16, tag="hsq")
                        hs_dma(t[:], hsb[l])
                        for c in range(N_CHUNKS):
                            lhsT = t[:, QUART_HS + c * 2 : QUART_HS + c * 2 + 2]
                            col0 = c * H
                            mm_args.append((lhsT, t[:, col0 : col0 + 512],
                                            t[:, col0 + 512 : col0 + H]))

                # Bank-A phase (N=512), then bank-B (N=256): the A copy
                # overlaps the B matmuls.
                psum_a = psum_pool.tile([2, 512], f32, tag="psum_a")
                psum_b = psum_pool.tile([2, H - 512], f32, tag="psum_b")
                n = len(mm_args)
                for i, (lhsT, rhs_a, _) in enumerate(mm_args):
                    nc.tensor.matmul(psum_a[:, :], lhsT, rhs_a,
                                     start=i == 0, stop=i == n - 1)
                nc.vector.tensor_copy(
                    out=out_tile[:, b * H : b * H + 512], in_=psum_a[:, :]
                )
                for i, (lhsT, _, rhs_b) in enumerate(mm_args):
                    nc.tensor.matmul(psum_b[:, :], lhsT, rhs_b,
                                     start=i == 0, stop=i == n - 1)
                nc.vector.tensor_copy(
                    out=out_tile[:, b * H + 512 : (b + 1) * H], in_=psum_b[:, :]
                )

            # SBUF [2, (b h)] -> DRAM [b, (m h)] in one 24KB DMA.
            # SWDGE (gpsimd): the 8 hs DMAs consume all 8 HWDGE sem lanes;
            # a 9th HWDGE DMA would wrap the lane and need a 2nd sync wait.
            nc.gpsimd.dma_start(
                out=out.rearrange("b (m h) -> m b h", m=2),
                in_=out_tile[:].rearrange("m (b h) -> m b h", b=B_SHARD),
            )

    _fix_drain_waits(nc)
    return nc


def _fix_drain_waits(nc):
    """This container's walrus accepts only ONE sync wait per instruction;
    Tile's exit drain aggregates one wait per live semaphore. In this kernel
    every semaphore except the final out-DMA's is transitively ordered before
    the drain (matmuls wait on hs DMAs -> PE; copies wait on PE -> DVE; the
    out DMA waits on DVE; the drain runs after on the same SP queue), so the
    drain only truly needs the out-DMA completion wait.
    """
    import bass_rust

    f = nc.m.functions[0]
    # update-sem of the last DMACopy in program order (the out store)
    last_dma_sem = None
    for bb in f.blocks:
        for ins in bb.instructions:
            if type(ins).__name__ == "InstDMACopy":
                ups = ins.sync_info.on_update
                if ups:
                    last_dma_sem = ups[-1].ant_name

    for bb in f.blocks:
        for ins in bb.instructions:
            if type(ins).__name__ != "InstDrain":
                continue
            si = ins.sync_info
            if si is None:
                continue
            waits = list(si.on_wait)
            if len(waits) <= 1:
                continue
            keep = [w for w in waits if w.ant_name == last_dma_sem]
            assert len(keep) == 1, (last_dma_sem, [w.ant_name for w in waits])
            ins.sync_info = bass_rust.SyncInfo(
                on_wait=keep, on_update=list(si.on_update)
            )


def _host_masks(input_ids, attention_mask, token_type_ids):
    ids = np.asarray(input_ids)
    am = np.asarray(attention_mask)
    tt = np.asarray(token_type_ids)

    not_pad = ids != PAD_ID
    before_pad = np.cumprod(not_pad.astype(np.int64), axis=1).astype(bool)
    valid = before_pad & (ids != CLS_ID) & (ids != SEP_ID) & (am == 1)
    term = valid & (tt == 0)
    text = valid & (tt == 1)
    masks = np.stack([term, text], axis=-1)  # [B, S, 2] bool
    counts = masks.sum(axis=1).astype(np.float64)  # [B, 2]
    return masks.astype(np.float16), counts


def kernel(hidden_states, input_ids, attention_mask, token_type_ids):
    from concourse.bass_utils import run_bass_kernel_spmd

    hs_full = np.asarray(hidden_states)
    masks, counts = _host_masks(input_ids, attention_mask, token_type_ids)

    hs4 = hs_full[L - N_LAYERS :].astype(np.float16)  # [4, B, S, H]

    # Half-blobs [B, hf, p, (l2 c h)] and quarter-blobs [B, l, p, (c h)]
    half = np.empty((B, 2, 128, HALF_COLS), dtype=np.float16)
    half[:, :, :, :HALF_HS] = (
        hs4.reshape(2, 2, B, N_CHUNKS, 128, H)
        .transpose(2, 0, 4, 1, 3, 5)
        .reshape(B, 2, 128, HALF_HS)
    )
    quart = np.empty((B, N_LAYERS, 128, QUART_COLS), dtype=np.float16)
    quart[:, :, :, :QUART_HS] = (
        hs4.reshape(N_LAYERS, B, N_CHUNKS, 128, H)
        .transpose(1, 0, 3, 2, 4)
        .reshape(B, N_LAYERS, 128, QUART_HS)
    )
    wv = masks.reshape(B, N_CHUNKS, 128, 2).transpose(0, 2, 1, 3).reshape(
        B, 128, W_COLS
    )
    half[:, :, :, HALF_HS:] = wv[:, None, :, :]
    quart[:, :, :, QUART_HS:] = wv[:, None, :, :]

    in_maps = [
        {
            "hsa": half[i * B_SHARD : i * B_SHARD + 3],
            "hsb": quart[i * B_SHARD + 3],
        }
        for i in range(N_CORES)
    ]

    if "nc" not in _CACHED:
        _CACHED["nc"] = _build_bass()
    nc = _CACHED["nc"]

    trace = os.environ.get("KERNEL_TRACE", "0") == "1"
    if trace:
        _install_ntff_hook_shim()
    tmpdir = os.environ.get("KERNEL_TMPDIR") or None
    res = run_bass_kernel_spmd(
        nc, in_maps, core_ids=list(range(N_CORES)), trace=trace, tmpdir=tmpdir
    )
    kernel.last_results = res

    acc = np.concatenate([r["out"] for r in res.results], axis=0)  # [B, 2H]
    # Apply the masked-mean normalization (exact f64 scale, mirrors the
    # reference's sum/count including inf/nan semantics for count==0).
    with np.errstate(divide="ignore", invalid="ignore"):
        scale = 1.0 / (N_LAYERS * counts)  # [B, 2]
    out = acc.reshape(B, 2, H) * scale[:, :, None]
    return out.reshape(B, 2 * H).astype(np.float32)


def _install_ntff_hook_shim():
    """The container's antenv stub lacks axon_hooks, which silently disables
    NTFF profiling under trace=True. Recreate it: a tiny get/set registry plus
    the ctypes hook into libaxon_pjrt.so (same as trn_boot's installer)."""
    import contextlib
    import ctypes
    import sys
    import types

    if "antenv.axon_hooks" in sys.modules:
        return
    so_path = "/opt/axon/libaxon_pjrt.so"
    try:
        lib = ctypes.CDLL(so_path)
    except OSError:
        return
    if not hasattr(lib, "axon_start_nrt_profile"):
        return
    lib.axon_start_nrt_profile.argtypes = [
        ctypes.POINTER(ctypes.c_int64),
        ctypes.c_size_t,
    ]
    lib.axon_start_nrt_profile.restype = ctypes.c_int64
    lib.axon_stop_nrt_profile.argtypes = [ctypes.c_char_p]
    lib.axon_stop_nrt_profile.restype = ctypes.c_int64

    @contextlib.contextmanager
    def _hook(output_dir, device_ids):
        import jax

        jax.devices()
        if device_ids:
            ids = (ctypes.c_int64 * len(device_ids))(*device_ids)
            rc = lib.axon_start_nrt_profile(ids, len(device_ids))
        else:
            rc = lib.axon_start_nrt_profile(None, 0)
        if rc != 0:
            raise RuntimeError(f"axon_start_nrt_profile rc={rc}")
        try:
            yield
        finally:
            n = lib.axon_stop_nrt_profile(str(output_dir).encode())
            print(f"profile: {n} file(s) written to {output_dir}", file=sys.stderr)

    mod = types.ModuleType("antenv.axon_hooks")
    _state = {"hook": _hook}
    mod.set_axon_ntff_profile_hook = lambda h: _state.__setitem__("hook", h)
    mod.get_axon_ntff_profile_hook = lambda: _state["hook"]
    sys.modules["antenv.axon_hooks"] = mod
    import antenv

    antenv.axon_hooks = mod


# revision 20
# speedup vs baseline: 1.1385x; 1.0187x over previous
"""Bass/Trainium2 kernel for nn_CustomPooling (segment_reduce, masked mean pooling).

Reference computation:
  hs = mean(hidden_states[-4:], axis=0)                      # [B,S,H]
  valid = before_pad & ~CLS & ~SEP & attention
  term_mean = sum_s(hs * term_mask) / sum(term_mask)         # [B,H]
  text_mean = sum_s(hs * text_mask) / sum(text_mask)         # [B,H]
  out = concat([term_mean, text_mean], -1)                   # [B,2H]

Strategy:
  - Only the last 4 layers are ever read (201MB of the 654MB input).
  - The [B,S] int masks reduce to binary {0,1} per-(b,s) weights; the
    1/(4*count) scale is applied to the tiny [B,2H] result on the host, so
    the device work is a pure masked sum over (layer, s):
      acc[b, m*H + h] = sum_{l,s} hs[l,b,s,h] * mask[b,s,m]
  - That reduction is a TensorE matmul with the [128,2] binary mask slice
    stationary and hs [128, N] moving, accumulated in fp32 PSUM over
    4 s-chunks x 4 layers. Data is shipped as fp16 ({0,1} masks are exact;
    hs quantization gives ~4e-4 rel err) which halves DMA bytes and runs
    the PE at full (1 col/cycle) rate instead of the 4x-slower fp32 path.
  - Data parallel over B: 8 cores x 4 batches, no collectives.
  - Host pre-swizzles each (batch, layer-pair) into one contiguous
    [128, 6152] fp16 blob (its own weight copy appended) so each tile is
    ONE ~1.57MB DMA and every matmul waits on exactly one DMA semaphore
    (this toolchain accepts a single sync wait per instruction). The 8 hs
    DMAs alternate between the two HWDGE rings (sync/scalar) to keep all
    16 SDMA engines latency-hidden; the tiny output store uses SWDGE to
    avoid wrapping the 8 HWDGE semaphore lanes.
"""

import os

import numpy as np

# Hardcoded problem shape (kernel.py must be self-contained).
L, B, S, H = 13, 32, 512, 768
N_LAYERS = 4          # layers -4..-1
N_CORES = 8
B_SHARD = B // N_CORES          # 4 batches per core
N_CHUNKS = S // 128             # 4 s-chunks of 128 (PE contraction dim)
W_COLS = N_CHUNKS * 2                    # 8
# Bulk batches (0..2) ship as two half-blobs (2 layers each); the tail
# batch ships as four quarter-blobs (1 layer) so the last-arriving tile
# only needs ~1.4us of matmuls after the final DMA lands.
HALF_HS = 2 * N_CHUNKS * H               # 6144
HALF_COLS = HALF_HS + W_COLS             # 6152
QUART_HS = N_CHUNKS * H                  # 3072
QUART_COLS = QUART_HS + W_COLS           # 3080
CLS_ID, SEP_ID, PAD_ID = 101, 102, 0

_CACHED = {}


def _build_bass():
    import concourse.bass as bass
    import concourse.tile as tile
    from concourse import mybir

    f16 = mybir.dt.float16
    f32 = mybir.dt.float32
    nc = bass.Bass()

    # Per-core inputs (host-preswizzled fp16 blobs, masks appended to each):
    #   hsa[b, hf, p, l2*3072 + c*768 + h], b in 0..2  (two half-blobs each)
    #   hsb[l, p, c*768 + h]                           (batch 3, per layer)
    hsa = nc.dram_tensor("hsa", [3, 2, 128, HALF_COLS], f16, kind="ExternalInput")
    hsb = nc.dram_tensor("hsb", [N_LAYERS, 128, QUART_COLS], f16, kind="ExternalInput")
    out = nc.dram_tensor("out", [B_SHARD, 2 * H], f32, kind="ExternalOutput")

    dma_idx = [0]

    def hs_dma(out_ap, in_ap):
        eng = nc.sync if dma_idx[0] % 2 == 0 else nc.scalar
        dma_idx[0] += 1
        eng.dma_start(out=out_ap, in_=in_ap)

    with tile.TileContext(nc) as tc:
        with (
            tc.tile_pool(name="hs_pool", bufs=6) as hs_pool,
            tc.tile_pool(name="hsq_pool", bufs=4) as hsq_pool,
            tc.tile_pool(name="out_pool", bufs=1) as out_pool,
            tc.tile_pool(name="psum", bufs=4, space="PSUM") as psum_pool,
        ):
            out_tile = out_pool.tile([2, B_SHARD * H], f32)

            for b in range(B_SHARD):
                # (lhsT, rhs_A, rhs_B) per (layer, chunk); weights live in
                # whichever tile the rhs comes from so each matmul waits on
                # exactly one DMA.
                mm_args = []
                if b < 3:
                    for hf in range(2):
                        t = hs_pool.tile([128, HALF_COLS], f16, tag="hs")
                        hs_dma(t[:], hsa[b, hf])
                        for l2 in range(2):
                            for c in range(N_CHUNKS):
                                lhsT = t[:, HALF_HS + c * 2 : HALF_HS + c * 2 + 2]
                                col0 = (l2 * N_CHUNKS + c) * H
                                mm_args.append((lhsT, t[:, col0 : col0 + 512],
                                                t[:, col0 + 512 : col0 + H]))
                else:
                    for l in range(N_LAYERS):
                        t = hsq_pool.tile([128, QUART_COLS], f16, tag="hsq")
                        hs_dma(t[:], hsb[l])
                        for c in range(N_CHUNKS):
                            lhsT = t[:, QUART_HS + c * 2 : QUART_HS + c * 2 + 2]
                            col0 = c * H
                            mm_args.append((lhsT, t[:, col0 : col0 + 512],
                                            t[:, col0 + 512 : col0 + H]))

                # Interleaved bank-A (N=512) / bank-B (N=256) groups in
                # separate PSUM banks; the A copy only waits on the A group
                # so it overlaps the final B matmul.
                psum_a = psum_pool.tile([2, 512], f32, tag="psum_a")
                psum_b = psum_pool.tile([2, H - 512], f32, tag="psum_b")
                n = len(mm_args)
                for i, (lhsT, rhs_a, rhs_b) in enumerate(mm_args):
                    nc.tensor.matmul(psum_a[:, :], lhsT, rhs_a,
                                     start=i == 0, stop=i == n - 1)
                    nc.tensor.matmul(psum_b[:, :], lhsT, rhs_b,
                                     start=i == 0, stop=i == n - 1)
                nc.vector.tensor_copy(
                    out=out_tile[:, b * H : b * H + 512], in_=psum_a[:, :]
                )
                nc.vector.tensor_copy(
                    out=out_tile[:, b * H + 512 : (b + 1) * H], in_=psum_b[:, :]
                )
                if b == 2:
                    # Bulk store (b0..b2) hides under b3's matmuls. Same
                    # SWDGE ring as the final store -> ring FIFO orders it
                    # before the final store's completion sem.
                    nc.gpsimd.dma_start(
                        out=out[0:3].rearrange("b (m h) -> m b h", m=2),
                        in_=out_tile[:, 0 : 3 * H].rearrange(
                            "m (b h) -> m b h", b=3
                        ),
                    )

            # Final (b3) store. SWDGE (gpsimd): the 10 hs DMAs wrap the 8
            # HWDGE sem lanes; more HWDGE DMAs would need a 2nd sync wait.
            nc.gpsimd.dma_start(
                out=out[3:4].rearrange("b (m h) -> m b h", m=2),
                in_=out_tile[:, 3 * H : 4 * H].rearrange(
                    "m (b h) -> m b h", b=1
                ),
            )

    _fix_drain_waits(nc)
    return nc


def _fix_drain_waits(nc):
    """This container's walrus accepts only ONE sync wait per instruction;
    Tile's exit drain aggregates one wait per live semaphore. In this kernel
    every semaphore except the final out-DMA's is transitively ordered before
    the drain (matmuls wait on hs DMAs -> PE; copies wait on PE -> DVE; the
    out DMA waits on DVE; the drain runs after on the same SP queue), so the
    drain only truly needs the out-DMA completion wait.
    """
    import bass_rust

    f = nc.m.functions[0]
    # update-sem of the last DMACopy in program order (the out store)
    last_dma_sem = None
    for bb in f.blocks:
        for ins in bb.instructions:
            if type(ins).__name__ == "InstDMACopy":
                ups = ins.sync_info.on_update
                if ups:
                    last_dma_sem = ups[-1].ant_name

    for bb in f.blocks:
        for ins in bb.instructions:
            if type(ins).__name__ != "InstDrain":
                continue
            si = ins.sync_info
            if si is None:
                continue
            waits = list(si.on_wait)
            if len(waits) <= 1:
                continue
            keep = [w for w in waits if w.ant_name == last_dma_sem]
            assert len(keep) == 1, (last_dma_sem, [w.ant_name for w in waits])
            ins.sync_info = bass_rust.SyncInfo(
                on_wait=keep, on_update=list(si.on_update)
            )


def _host_masks(input_ids, attention_mask, token_type_ids):
    ids = np.asarray(input_ids)
    am = np.asarray(attention_mask)
    tt = np.asarray(token_type_ids)

    not_pad = ids != PAD_ID
    before_pad = np.cumprod(not_pad.astype(np.int64), axis=1).astype(bool)
    valid = before_pad & (ids != CLS_ID) & (ids != SEP_ID) & (am == 1)
    term = valid & (tt == 0)
    text = valid & (tt == 1)
    masks = np.stack([term, text], axis=-1)  # [B, S, 2] bool
    counts = masks.sum(axis=1).astype(np.float64)  # [B, 2]
    return masks.astype(np.float16), counts


def kernel(hidden_states, input_ids, attention_mask, token_type_ids):
    from concourse.bass_utils import run_bass_kernel_spmd

    hs_full = np.asarray(hidden_states)
    masks, counts = _host_masks(input_ids, attention_mask, token_type_ids)

    hs4 = hs_full[L - N_LAYERS :].astype(np.float16)  # [4, B, S, H]

    # Half-blobs [B, hf, p, (l2 c h)] and quarter-blobs [B, l, p, (c h)]
    half = np.empty((B, 2, 128, HALF_COLS), dtype=np.float16)
    half[:, :, :, :HALF_HS] = (
        hs4.reshape(2, 2, B, N_CHUNKS, 128, H)
        .transpose(2, 0, 4, 1, 3, 5)
        .reshape(B, 2, 128, HALF_HS)
    )
    quart = np.empty((B, N_LAYERS, 128, QUART_COLS), dtype=np.float16)
    quart[:, :, :, :QUART_HS] = (
        hs4.reshape(N_LAYERS, B, N_CHUNKS, 128, H)
        .transpose(1, 0, 3, 2, 4)
        .reshape(B, N_LAYERS, 128, QUART_HS)
    )
    wv = masks.reshape(B, N_CHUNKS, 128, 2).transpose(0, 2, 1, 3).reshape(
        B, 128, W_COLS
    )
    half[:, :, :, HALF_HS:] = wv[:, None, :, :]
    quart[:, :, :, QUART_HS:] = wv[:, None, :, :]

    in_maps = [
        {
            "hsa": half[i * B_SHARD : i * B_SHARD + 3],
            "hsb": quart[i * B_SHARD + 3],
        }
        for i in range(N_CORES)
    ]

    if "nc" not in _CACHED:
        _CACHED["nc"] = _build_bass()
    nc = _CACHED["nc"]

    trace = os.environ.get("KERNEL_TRACE", "0") == "1"
    if trace:
        _install_ntff_hook_shim()
    tmpdir = os.environ.get("KERNEL_TMPDIR") or None
    res = run_bass_kernel_spmd(
        nc, in_maps, core_ids=list(range(N_CORES)), trace=trace, tmpdir=tmpdir
    )
    kernel.last_results = res

    acc = np.concatenate([r["out"] for r in res.results], axis=0)  # [B, 2H]
    # Apply the masked-mean normalization (exact f64 scale, mirrors the
    # reference's sum/count including inf/nan semantics for count==0).
    with np.errstate(divide="ignore", invalid="ignore"):
        scale = 1.0 / (N_LAYERS * counts)  # [B, 2]
    out = acc.reshape(B, 2, H) * scale[:, :, None]
    return out.reshape(B, 2 * H).astype(np.float32)


def _install_ntff_hook_shim():
    """The container's antenv stub lacks axon_hooks, which silently disables
    NTFF profiling under trace=True. Recreate it: a tiny get/set registry plus
    the ctypes hook into libaxon_pjrt.so (same as trn_boot's installer)."""
    import contextlib
    import ctypes
    import sys
    import types

    if "antenv.axon_hooks" in sys.modules:
        return
    so_path = "/opt/axon/libaxon_pjrt.so"
    try:
        lib = ctypes.CDLL(so_path)
    except OSError:
        return
    if not hasattr(lib, "axon_start_nrt_profile"):
        return
    lib.axon_start_nrt_profile.argtypes = [
        ctypes.POINTER(ctypes.c_int64),
        ctypes.c_size_t,
    ]
    lib.axon_start_nrt_profile.restype = ctypes.c_int64
    lib.axon_stop_nrt_profile.argtypes = [ctypes.c_char_p]
    lib.axon_stop_nrt_profile.restype = ctypes.c_int64

    @contextlib.contextmanager
    def _hook(output_dir, device_ids):
        import jax

        jax.devices()
        if device_ids:
            ids = (ctypes.c_int64 * len(device_ids))(*device_ids)
            rc = lib.axon_start_nrt_profile(ids, len(device_ids))
        else:
            rc = lib.axon_start_nrt_profile(None, 0)
        if rc != 0:
            raise RuntimeError(f"axon_start_nrt_profile rc={rc}")
        try:
            yield
        finally:
            n = lib.axon_stop_nrt_profile(str(output_dir).encode())
            print(f"profile: {n} file(s) written to {output_dir}", file=sys.stderr)

    mod = types.ModuleType("antenv.axon_hooks")
    _state = {"hook": _hook}
    mod.set_axon_ntff_profile_hook = lambda h: _state.__setitem__("hook", h)
    mod.get_axon_ntff_profile_hook = lambda: _state["hook"]
    sys.modules["antenv.axon_hooks"] = mod
    import antenv

    antenv.axon_hooks = mod
